# revision 12
# baseline (speedup 1.0000x reference)
"""Binarized CNN forward pass on 8 TRN2 NeuronCores (data-parallel, batch 256).

Self-contained: kernel(**inputs) takes the full unsharded inputs (as produced
by the reference's setup_inputs) and returns the full [256, 10] output.

Strategy
--------
- Pure data parallelism: 32 images per core; binarized (+-1) weights are exact
  in bf16 and replicated to all cores.
- All matmuls run in bf16 with a hi/lo activation decomposition
  (x = bf16(x) + bf16(x - bf16(x)), weights exact) accumulated in fp32 PSUM:
  ~2^-16 effective activation precision at 2 PE cycles/row.
- Training-mode BN needs full-batch statistics: per conv layer each core
  computes per-channel (count, mean, M2) via bn_stats/bn_aggr, cores
  all-reduce (mean, var+mean^2), and the affine relu(a*x+c) is applied with
  per-partition a, c on the scalar engine.
- maxpool commutes with the monotone BN+relu transform (a > 0), so raw conv
  outputs are pooled first and transformed after.
- Activation layout: [c_chunk(128 partitions), h, w, b] with b=32 innermost.
- Raw conv outputs of layers 3/4 do not fit in SBUF next to their inputs and
  are spilled through DRAM.
- SBUF pools: activations live on the left stack (one layer at a time),
  per-layer transients (weights/stats/raw/y32) on the right stack, released
  LIFO at the end of each layer.
- Layer 1 (cin=1) uses an im2col with K=9 taps on partitions, packed 4x into
  PE row-groups (tile_position) over quarters of the output rows.
"""
import numpy as np
import ml_dtypes
from concourse import bacc, tile, mybir
from concourse.ap import AP
from concourse.bass_utils import run_bass_kernel_spmd

BF16 = mybir.dt.bfloat16
F32 = mybir.dt.float32
AF = mybir.ActivationFunctionType
ALU = mybir.AluOpType

N_CORES = 8
B = 32          # per-core batch
EPS = 1e-5

# l, cin, cout, hin (after any pool of prev layer), hout, pool_after
CONV = [
    (1, 1, 128, 28, 26, False),
    (2, 128, 128, 26, 24, False),
    (3, 128, 256, 24, 22, False),
    (4, 256, 256, 22, 20, True),
    (5, 256, 512, 10, 8, False),
    (6, 512, 512, 8, 6, True),
]
SPILL = {3, 4}

# layer-1 output-row groups for 4x PE row-group packing
L1_GROUPS = [(0, 7), (7, 6), (13, 7), (20, 6)]


def _halves(wout):
    """Split a row of wout*B output columns into <=512-col matmul chunks."""
    n = wout * B
    if n <= 512:
        return [(0, wout)]
    assert wout % 2 == 0
    return [(0, wout // 2), (wout // 2, wout // 2)]


def build(upto=9):
    """Build the Bass module. upto: 1..6 = stop after conv layer `upto` and
    emit its transformed activations as debug outputs; 9 = full net."""
    nc = bacc.Bacc("TRN2", target_bir_lowering=False, debug=False,
                   num_devices=N_CORES)

    # ---- parameters (per-core shards / replicated weights)
    x_hi = nc.declare_dram_parameter("x_hi", [28 * 28, B], BF16, isOutput=False)
    x_lo = nc.declare_dram_parameter("x_lo", [28 * 28, B], BF16, isOutput=False)
    w1p = nc.declare_dram_parameter("w1p", [128, 128], BF16, isOutput=False)
    wp = {}
    gp, bp = {}, {}
    for (l, cin, cout, hin, hout, pool) in CONV:
        if l >= 2:
            ci_ch, co_ch = cin // 128, cout // 128
            wp[l] = nc.declare_dram_parameter(
                f"w{l}p", [ci_ch * 128, co_ch * 9 * 128], BF16, isOutput=False)
        gp[l] = nc.declare_dram_parameter(f"g{l}p", [cout, 1], F32, isOutput=False)
        bp[l] = nc.declare_dram_parameter(f"b{l}p", [cout, 1], F32, isOutput=False)
    fw1p = fw2p = fw3p = None
    if upto >= 9:
        fw1p = nc.declare_dram_parameter("fw1p", [36 * 128, 1024], BF16, isOutput=False)
        fw2p = nc.declare_dram_parameter("fw2p", [8 * 128, 1024], BF16, isOutput=False)
        fw3p = nc.declare_dram_parameter("fw3p", [8 * 128, 10], BF16, isOutput=False)
        out_ext = nc.declare_dram_parameter("out", [10, B], F32, isOutput=True)

    dbg_hi = dbg_lo = None
    if upto < 9:
        (_, _, cout, _, hout, pool) = CONV[upto - 1]
        ho = hout // 2 if pool else hout
        co_ch = cout // 128
        dbg_hi = nc.declare_dram_parameter(
            "dbg_hi", [co_ch * 128, ho * ho * B], BF16, isOutput=True)
        dbg_lo = nc.declare_dram_parameter(
            "dbg_lo", [co_ch * 128, ho * ho * B], BF16, isOutput=True)

    # ---- DRAM scratch
    cc_in, cc_out = {}, {}
    for (l, cin, cout, hin, hout, pool) in CONV:
        if l > upto:
            break
        co_ch = cout // 128
        cc_in[l] = nc.dram_tensor(f"cc_in{l}", [128, 2 * co_ch], F32)
        cc_out[l] = nc.dram_tensor(f"cc_out{l}", [128, 2 * co_ch], F32,
                                   addr_space="Shared")
    raw_dram = {}
    for l in SPILL:
        if l > upto:
            continue
        (_, cin, cout, hin, hout, pool) = CONV[l - 1]
        co_ch = cout // 128
        raw_dram[l] = nc.dram_tensor(f"raw{l}d", [co_ch * hout * 128, hout * B], F32)

    with tile.TileContext(nc) as tc:
        const_pool = tc.alloc_tile_pool(name="const", bufs=1, side="left")
        stage_pool = tc.alloc_tile_pool(name="stage", bufs=1, side="right")
        psum_pool = tc.alloc_tile_pool(name="psum", bufs=6, space="PSUM")

        eps_t = const_pool.tile([128, 1], F32, tag="eps")
        nc.vector.memset(eps_t[:], EPS)

        # per-layer gamma/beta -> SBUF  [128,1] per chunk
        gb_sb = {}
        for (l, cin, cout, hin, hout, pool) in CONV:
            if l > upto:
                break
            co_ch = cout // 128
            for co in range(co_ch):
                gt = const_pool.tile([128, 1], F32, tag=f"g{l}_{co}")
                bt = const_pool.tile([128, 1], F32, tag=f"b{l}_{co}")
                nc.sync.dma_start(out=gt[:], in_=gp[l][co * 128:(co + 1) * 128, :])
                nc.sync.dma_start(out=bt[:], in_=bp[l][co * 128:(co + 1) * 128, :])
                gb_sb[(l, co)] = (gt, bt)

        # ---------------- conv stack ----------------
        def conv_layer(l, act_in, act_pool_in):
            """act_in: dict (ci, row) -> (hi_tile, lo_tile) of input rows
            [128, win*B] (layer 1: the packed im2col tiles under key (0, 0)).
            Returns (act_out, act_pool_out)."""
            (_, cin, cout, hin, hout, pool) = CONV[l - 1]
            ci_ch, co_ch = (cin // 128, cout // 128) if l >= 2 else (1, 1)
            spill = l in SPILL
            halves = _halves(hout)
            ntiles = hout * len(halves)

            # ---- right-stack transients (alloc order = reverse release order)
            wpool = tc.alloc_tile_pool(name=f"w{l}", bufs=1, side="right")
            stats_pool = tc.alloc_tile_pool(name=f"st{l}", bufs=1, side="right")
            raw_pool = None
            if not spill:
                raw_pool = tc.alloc_tile_pool(name=f"raw{l}", bufs=1, side="right")
            y32_pool = tc.alloc_tile_pool(name=f"y32_{l}", bufs=3, side="right")

            w_sb = {}
            if l >= 2:
                for ci in range(ci_ch):
                    for co in range(co_ch):
                        t = wpool.tile([128, 9 * 128], BF16, tag=f"w{ci}_{co}",
                                       name=f"w{l}_{ci}_{co}")
                        nc.sync.dma_start(
                            out=t[:],
                            in_=wp[l][ci * 128:(ci + 1) * 128,
                                      co * 1152:(co + 1) * 1152])
                        w_sb[(ci, co)] = t
            else:
                t = wpool.tile([128, 128], BF16, tag="w1", name="w1sb")
                nc.sync.dma_start(out=t[:], in_=w1p[:, :])
                w_sb[(0, 0)] = t

            stats = {}
            for co in range(co_ch):
                stats[co] = stats_pool.tile([128, ntiles * 6], F32, tag=f"s{co}",
                                            name=f"stats{l}_{co}")
            raw = {}
            if not spill:
                for co in range(co_ch):
                    for r in range(hout):
                        raw[(co, r)] = raw_pool.tile([128, hout * B], F32,
                                                     tag=f"r{co}_{r}",
                                                     name=f"raw{l}_{co}_{r}")

            def evac(ps, co, r, hidx, w0, wn):
                n = wn * B
                tidx = r * len(halves) + hidx
                nc.vector.bn_stats(
                    out=stats[co][:, tidx * 6:(tidx + 1) * 6], in_=ps[:])
                if spill:
                    sg = stage_pool.tile([128, n], F32, tag="evac", bufs=4,
                                         name="evac_sg")
                    nc.scalar.copy(out=sg[:], in_=ps[:])
                    nc.sync.dma_start(
                        out=raw_dram[l][(co * hout + r) * 128:
                                        (co * hout + r + 1) * 128,
                                        w0 * B:w0 * B + n],
                        in_=sg[:])
                else:
                    nc.scalar.copy(out=raw[(co, r)][:, w0 * B:w0 * B + n],
                                   in_=ps[:])

            # ---- matmuls
            if l == 1:
                ic_hi, ic_lo = act_in[(0, 0)]
                for gi, (r0, nr) in enumerate(L1_GROUPS):
                    for rr in range(nr):
                        for hidx, (w0, wn) in enumerate(halves):
                            n = wn * B
                            ps = psum_pool.tile([128, n], F32, tag="ps",
                                                name="ps1")
                            off = (rr * 26 + w0) * B
                            lhsT = w_sb[(0, 0)][32 * gi:32 * gi + 9, :]
                            nc.tensor.matmul(
                                out=ps[:], lhsT=lhsT,
                                rhs=ic_hi[32 * gi:32 * gi + 9, off:off + n],
                                start=True, stop=False,
                                tile_position=(32 * gi, 0))
                            nc.tensor.matmul(
                                out=ps[:], lhsT=lhsT,
                                rhs=ic_lo[32 * gi:32 * gi + 9, off:off + n],
                                start=False, stop=True,
                                tile_position=(32 * gi, 0))
                            evac(ps, 0, r0 + rr, hidx, w0, wn)
            else:
                for r in range(hout):
                    for co in range(co_ch):
                        for hidx, (w0, wn) in enumerate(halves):
                            n = wn * B
                            ps = psum_pool.tile([128, n], F32, tag="ps",
                                                name="psc")
                            nmm = ci_ch * 9 * 2
                            k = 0
                            for ci in range(ci_ch):
                                for dy in range(3):
                                    hi_t, lo_t = act_in[(ci, r + dy)]
                                    v_hi = hi_t[:].rearrange(
                                        "p (w b) -> p w b", b=B)
                                    v_lo = lo_t[:].rearrange(
                                        "p (w b) -> p w b", b=B)
                                    for dx in range(3):
                                        rhs_hi = v_hi[:, w0 + dx:w0 + dx + wn, :]
                                        rhs_lo = v_lo[:, w0 + dx:w0 + dx + wn, :]
                                        t = dy * 3 + dx
                                        lhsT = w_sb[(ci, co)][
                                            :, t * 128:(t + 1) * 128]
                                        nc.tensor.matmul(
                                            out=ps[:], lhsT=lhsT, rhs=rhs_hi,
                                            start=(k == 0), stop=False)
                                        k += 1
                                        nc.tensor.matmul(
                                            out=ps[:], lhsT=lhsT, rhs=rhs_lo,
                                            start=False, stop=(k == nmm - 1))
                                        k += 1
                            evac(ps, co, r, hidx, w0, wn)

            # ---- stats -> allreduce -> a, c
            cc_sb = const_pool.tile([128, 2 * co_ch], F32, tag=f"cc{l}",
                                    name=f"cc{l}")
            for co in range(co_ch):
                nc.vector.bn_aggr(out=cc_sb[:, co * 2:co * 2 + 2], in_=stats[co][:])
                # replace var slot with var + mean^2
                nc.vector.scalar_tensor_tensor(
                    out=cc_sb[:, co * 2 + 1:co * 2 + 2],
                    in0=cc_sb[:, co * 2:co * 2 + 1],
                    scalar=cc_sb[:, co * 2:co * 2 + 1],
                    in1=cc_sb[:, co * 2 + 1:co * 2 + 2],
                    op0=ALU.mult, op1=ALU.add)
            nc.sync.dma_start(out=cc_in[l][:, :], in_=cc_sb[:])
            nc.gpsimd.collective_compute(
                "AllReduce", ALU.add, replica_groups=[list(range(N_CORES))],
                ins=[cc_in[l][:, :]], outs=[cc_out[l][:, :]])
            cc_g = const_pool.tile([128, 2 * co_ch], F32, tag=f"ccg{l}",
                                   name=f"ccg{l}")
            nc.sync.dma_start(out=cc_g[:], in_=cc_out[l][:, :])

            ac = {}
            for co in range(co_ch):
                gt, bt = gb_sb[(l, co)]
                nm = const_pool.tile([128, 1], F32, tag=f"nm{l}_{co}",
                                     name=f"nm{l}_{co}")
                # nm = -mean_global = s1 * (-1/8)
                nc.scalar.mul(out=nm[:], in_=cc_g[:, co * 2:co * 2 + 1],
                              mul=-1.0 / N_CORES)
                nvar = const_pool.tile([128, 1], F32, tag=f"va{l}_{co}",
                                       name=f"nvar{l}_{co}")
                # nvar = mean^2 - s2/8 = -var
                nc.scalar.mul(out=nvar[:], in_=cc_g[:, co * 2 + 1:co * 2 + 2],
                              mul=1.0 / N_CORES)
                nc.vector.scalar_tensor_tensor(
                    out=nvar[:], in0=nm[:], scalar=nm[:], in1=nvar[:],
                    op0=ALU.mult, op1=ALU.subtract)
                sd = const_pool.tile([128, 1], F32, tag=f"sd{l}_{co}",
                                     name=f"sd{l}_{co}")
                nc.scalar.activation(out=sd[:], in_=nvar[:], func=AF.Sqrt,
                                     bias=eps_t[:], scale=-1.0)
                rc = const_pool.tile([128, 1], F32, tag=f"rc{l}_{co}",
                                     name=f"rc{l}_{co}")
                nc.vector.reciprocal(out=rc[:], in_=sd[:])
                a_t = const_pool.tile([128, 1], F32, tag=f"a{l}_{co}",
                                      name=f"a{l}_{co}")
                nc.vector.tensor_mul(out=a_t[:], in0=rc[:], in1=gt[:])
                c_t = const_pool.tile([128, 1], F32, tag=f"c{l}_{co}",
                                      name=f"c{l}_{co}")
                # c = b + nm * a
                nc.vector.scalar_tensor_tensor(
                    out=c_t[:], in0=nm[:], scalar=a_t[:], in1=bt[:],
                    op0=ALU.mult, op1=ALU.add)
                ac[co] = (a_t, c_t)

            # ---- input activations are dead; swap left-stack pools
            if act_pool_in is not None:
                act_pool_in.release()
            act_pool_out = tc.alloc_tile_pool(name=f"act{l}", bufs=1,
                                              side="left")

            # ---- transform (+ optional pool)
            ho = hout // 2 if pool else hout
            wo = hout // 2 if pool else hout
            act_out = {}
            for co in range(co_ch):
                a_t, c_t = ac[co]
                for r in range(ho):
                    r0 = r1 = src = None
                    if spill:
                        if pool:
                            r0 = stage_pool.tile([128, hout * B], F32,
                                                 tag="rb0", bufs=2, name="rb0")
                            r1 = stage_pool.tile([128, hout * B], F32,
                                                 tag="rb1", bufs=2, name="rb1")
                            nc.sync.dma_start(
                                out=r0[:], in_=raw_dram[l][
                                    (co * hout + 2 * r) * 128:
                                    (co * hout + 2 * r + 1) * 128, :])
                            nc.sync.dma_start(
                                out=r1[:], in_=raw_dram[l][
                                    (co * hout + 2 * r + 1) * 128:
                                    (co * hout + 2 * r + 2) * 128, :])
                        else:
                            r0 = stage_pool.tile([128, hout * B], F32,
                                                 tag="rb0", bufs=2, name="rb0")
                            nc.sync.dma_start(
                                out=r0[:], in_=raw_dram[l][
                                    (co * hout + r) * 128:
                                    (co * hout + r + 1) * 128, :])
                            src = r0
                    else:
                        if pool:
                            r0, r1 = raw[(co, 2 * r)], raw[(co, 2 * r + 1)]
                        else:
                            src = raw[(co, r)]
                    if pool:
                        pm = stage_pool.tile([128, hout * B], F32, tag="pm",
                                             bufs=2, name="pm")
                        nc.vector.tensor_max(out=pm[:], in0=r0[:], in1=r1[:])
                        v = pm[:].rearrange("p (w two b) -> p w two b",
                                            two=2, b=B)
                        pr = stage_pool.tile([128, wo * B], F32, tag="pr",
                                             bufs=2, name="pr")
                        prv = pr[:].rearrange("p (w b) -> p w b", b=B)
                        nc.vector.tensor_max(out=prv[:, :, :],
                                             in0=v[:, :, 0, :],
                                             in1=v[:, :, 1, :])
                        src = pr
                    hi_t = act_pool_out.tile([128, wo * B], BF16,
                                             tag=f"h{co}_{r}",
                                             name=f"a{l}h_{co}_{r}")
                    lo_t = act_pool_out.tile([128, wo * B], BF16,
                                             tag=f"l{co}_{r}",
                                             name=f"a{l}l_{co}_{r}")
                    nc.scalar.activation(out=hi_t[:], in_=src[:], func=AF.Relu,
                                         bias=c_t[:], scale=a_t[:])
                    y32 = y32_pool.tile([128, wo * B], F32, tag="y32",
                                        name="y32t")
                    nc.scalar.activation(out=y32[:], in_=src[:], func=AF.Relu,
                                         bias=c_t[:], scale=a_t[:])
                    nc.vector.tensor_sub(out=lo_t[:], in0=y32[:], in1=hi_t[:])
                    act_out[(co, r)] = (hi_t, lo_t)

            # ---- pop right-stack transients (LIFO)
            y32_pool.release()
            if raw_pool is not None:
                raw_pool.release()
            stats_pool.release()
            wpool.release()
            return act_out, act_pool_out

        # ---- layer 1 im2col source (4 row-groups packed on partitions)
        im2col_pool = tc.alloc_tile_pool(name="im2col", bufs=1, side="left")
        ic_hi = im2col_pool.tile([128, 7 * 26 * B], BF16, tag="ic_hi")
        ic_lo = im2col_pool.tile([128, 7 * 26 * B], BF16, tag="ic_lo")
        for src, dst in ((x_hi, ic_hi), (x_lo, ic_lo)):
            for gi, (r0, nr) in enumerate(L1_GROUPS):
                for dy in range(3):
                    # partitions 32*gi + 3*dy + dx <- x[(r0+ho+dy)*28 + wo+dx, b]
                    in_ap = AP(src, ((r0 + dy) * 28) * B,
                               [[B, 3], [28 * B, nr], [B, 26], [1, B]])
                    nc.sync.dma_start(
                        out=dst[32 * gi + 3 * dy:32 * gi + 3 * dy + 3,
                                0:nr * 26 * B],
                        in_=in_ap)

        act = {(0, 0): (ic_hi, ic_lo)}
        act, act_pool = conv_layer(1, act, im2col_pool)

        for l in range(2, min(upto, 6) + 1):
            act, act_pool = conv_layer(l, act, act_pool)

        if upto < 9:
            # dump act (hi, lo) of last computed layer
            (_, _, cout, _, hout, pool) = CONV[upto - 1]
            ho = hout // 2 if pool else hout
            co_ch = cout // 128
            for co in range(co_ch):
                for r in range(ho):
                    hi_t, lo_t = act[(co, r)]
                    nc.sync.dma_start(
                        out=dbg_hi[co * 128:(co + 1) * 128,
                                   r * ho * B:(r + 1) * ho * B],
                        in_=hi_t[:])
                    nc.sync.dma_start(
                        out=dbg_lo[co * 128:(co + 1) * 128,
                                   r * ho * B:(r + 1) * ho * B],
                        in_=lo_t[:])
        else:
            # ---------------- FC stack ----------------
            fc_pool = tc.alloc_tile_pool(name="fc", bufs=1, side="right")
            fw1_sb = []
            for kc in range(36):
                t = fc_pool.tile([128, 1024], BF16, tag=f"fw1_{kc}",
                                 name=f"fw1_{kc}")
                nc.sync.dma_start(out=t[:],
                                  in_=fw1p[kc * 128:(kc + 1) * 128, :])
                fw1_sb.append(t)
            fw2_sb = []
            for kc in range(8):
                t = fc_pool.tile([128, 1024], BF16, tag=f"fw2_{kc}",
                                 name=f"fw2_{kc}")
                nc.sync.dma_start(out=t[:],
                                  in_=fw2p[kc * 128:(kc + 1) * 128, :])
                fw2_sb.append(t)
            fw3_sb = []
            for kc in range(8):
                t = fc_pool.tile([128, 10], BF16, tag=f"fw3_{kc}",
                                 name=f"fw3_{kc}")
                nc.sync.dma_start(out=t[:],
                                  in_=fw3p[kc * 128:(kc + 1) * 128, :])
                fw3_sb.append(t)

            def fc_relu_layer(w_sb, rhs_list, n_out_ch, tagp):
                """rhs_list: list over kc of (hi_ap, lo_ap) [128, B].
                Returns list over o_ch of (hi, lo) tiles [128, B]."""
                outs = []
                nkc = len(rhs_list)
                for o_ch in range(n_out_ch):
                    ps = psum_pool.tile([128, B], F32, tag="ps", name="psfc")
                    for kc, (rh, rl) in enumerate(rhs_list):
                        lhsT = w_sb[kc][:, o_ch * 128:(o_ch + 1) * 128]
                        nc.tensor.matmul(out=ps[:], lhsT=lhsT, rhs=rh,
                                         start=(kc == 0), stop=False)
                        nc.tensor.matmul(out=ps[:], lhsT=lhsT, rhs=rl,
                                         start=False, stop=(kc == nkc - 1))
                    hi_t = fc_pool.tile([128, B], BF16, tag=f"{tagp}h{o_ch}",
                                        name=f"{tagp}h{o_ch}")
                    lo_t = fc_pool.tile([128, B], BF16, tag=f"{tagp}l{o_ch}",
                                        name=f"{tagp}l{o_ch}")
                    nc.scalar.activation(out=hi_t[:], in_=ps[:], func=AF.Relu)
                    y32 = fc_pool.tile([128, B], F32, tag=f"{tagp}y{o_ch}",
                                       name=f"{tagp}y{o_ch}")
                    nc.scalar.activation(out=y32[:], in_=ps[:], func=AF.Relu)
                    nc.vector.tensor_sub(out=lo_t[:], in0=y32[:], in1=hi_t[:])
                    outs.append((hi_t, lo_t))
                return outs

            # fc1 rhs: act6 [(co,r)] tiles [128, 3*B]; kc = (co*3 + r)*3 + w
            rhs1 = []
            for co in range(4):
                for r in range(3):
                    hi_t, lo_t = act[(co, r)]
                    for w in range(3):
                        rhs1.append((hi_t[:, w * B:(w + 1) * B],
                                     lo_t[:, w * B:(w + 1) * B]))
            y1 = fc_relu_layer(fw1_sb, rhs1, 8, "y1")
            y2 = fc_relu_layer(fw2_sb, [(h[:], l_[:]) for h, l_ in y1], 8, "y2")

            ps = psum_pool.tile([10, B], F32, tag="ps", name="ps10")
            for kc, (rh, rl) in enumerate([(h[:], l_[:]) for h, l_ in y2]):
                lhsT = fw3_sb[kc][:, :]
                nc.tensor.matmul(out=ps[:], lhsT=lhsT, rhs=rh,
                                 start=(kc == 0), stop=False)
                nc.tensor.matmul(out=ps[:], lhsT=lhsT, rhs=rl,
                                 start=False, stop=(kc == 7))
            sig = fc_pool.tile([10, B], F32, tag="sig", name="sig")
            nc.scalar.activation(out=sig[:], in_=ps[:], func=AF.Sigmoid)
            nc.sync.dma_start(out=out_ext[:, :], in_=sig[:])
            fc_pool.release()

        act_pool.release()
        psum_pool.release()
        stage_pool.release()
        const_pool.release()

    nc.compile()
    return nc


# ---------------- host-side input prep ----------------

def _bf16(x):
    return np.asarray(x, np.float32).astype(ml_dtypes.bfloat16)


def prep_inputs(inputs):
    """Full reference inputs -> per-core input maps."""
    x = np.asarray(inputs["x"], np.float32).reshape(256, 28 * 28)
    shared = {}
    w1 = np.sign(np.asarray(inputs["w1"], np.float32)).reshape(128, 9)
    w1rep = np.zeros((128, 128), np.float32)
    for gi in range(4):
        w1rep[32 * gi:32 * gi + 9, :] = w1.T
    shared["w1p"] = _bf16(w1rep)
    for l in (2, 3, 4, 5, 6):
        w = np.sign(np.asarray(inputs[f"w{l}"], np.float32))
        cout, cin = w.shape[0], w.shape[1]
        ci_ch, co_ch = cin // 128, cout // 128
        s = w.reshape(co_ch, 128, ci_ch, 128, 9)
        s = np.transpose(s, (2, 3, 0, 4, 1))  # [ci_ch,128,co_ch,9,128]
        shared[f"w{l}p"] = _bf16(s.reshape(ci_ch * 128, co_ch * 9 * 128).copy())
    for l in range(1, 7):
        shared[f"g{l}p"] = np.asarray(inputs[f"g{l}"], np.float32).reshape(-1, 1)
        shared[f"b{l}p"] = np.asarray(inputs[f"b{l}"], np.float32).reshape(-1, 1)
    fw1 = np.sign(np.asarray(inputs["fw1"], np.float32))  # [1024, 4608]
    v = fw1.reshape(1024, 4, 128, 9)
    v = np.transpose(v, (1, 3, 2, 0))  # [4, 9, 128, 1024], kc = cc*9+hw
    shared["fw1p"] = _bf16(v.reshape(36 * 128, 1024).copy())
    fw2 = np.sign(np.asarray(inputs["fw2"], np.float32))
    shared["fw2p"] = _bf16(fw2.T.reshape(8 * 128, 1024).copy())
    fw3 = np.sign(np.asarray(inputs["fw3"], np.float32))
    shared["fw3p"] = _bf16(fw3.T.reshape(8 * 128, 10).copy())

    per_core = []
    for c in range(N_CORES):
        xs = x[c * B:(c + 1) * B].T.copy()  # [784, B]
        hi = _bf16(xs)
        lo = _bf16(xs - hi.astype(np.float32))
        m = dict(shared)
        m["x_hi"] = hi
        m["x_lo"] = lo
        per_core.append(m)
    return per_core


_NC_CACHE = {}


def kernel(**inputs) -> np.ndarray:
    if "nc" not in _NC_CACHE:
        _NC_CACHE["nc"] = build(upto=9)
    nc = _NC_CACHE["nc"]
    per_core = prep_inputs(inputs)
    res = run_bass_kernel_spmd(nc, per_core, list(range(N_CORES)))
    out = np.empty((256, 10), np.float32)
    for c in range(N_CORES):
        out[c * B:(c + 1) * B] = res.results[c]["out"].T
    return out


if __name__ == "__main__":
    import reference as R
    inputs = R.setup_inputs()
    got = kernel(**{k: np.asarray(v) for k, v in inputs.items()})
    exp = np.asarray(R.reference(**inputs))
    err = np.abs(got - exp)
    rel = np.linalg.norm(got - exp) / np.linalg.norm(exp)
    print(f"absmax {err.max():.3e}  rel {rel:.3e}")


# revision 17
# speedup vs baseline: 1.5111x; 1.5111x over previous
"""Binarized CNN forward pass on 8 TRN2 NeuronCores (data-parallel, batch 256).

Self-contained: kernel(**inputs) takes the full unsharded inputs (as produced
by the reference's setup_inputs) and returns the full [256, 10] output.

Strategy
--------
- Pure data parallelism: 32 images per core; binarized (+-1) weights are exact
  in bf16 and replicated to all cores.
- All matmuls run in bf16 with a hi/lo activation decomposition
  (x = bf16(x) + bf16(x - bf16(x)), weights exact) accumulated in fp32 PSUM:
  ~2^-16 effective activation precision at 2 PE cycles/row.
- Training-mode BN needs full-batch statistics: per conv layer each core
  computes per-channel (count, mean, M2) via bn_stats/bn_aggr, cores
  all-reduce (mean, var+mean^2), and the affine relu(a*x+c) is applied with
  per-partition a, c on the scalar engine.
- maxpool commutes with the monotone BN+relu transform (a > 0), so raw conv
  outputs are pooled first and transformed after.
- Activation layout: [c_chunk(128 partitions), h, w, b] with b=32 innermost.
- Raw conv outputs of layers 3/4 do not fit in SBUF next to their inputs and
  are spilled through DRAM.
- SBUF pools: activations live on the left stack (one layer at a time),
  per-layer transients (weights/stats/raw/y32) on the right stack, released
  LIFO at the end of each layer.
- Layer 1 (cin=1) uses an im2col with K=9 taps on partitions, packed 4x into
  PE row-groups (tile_position) over quarters of the output rows.
"""
import numpy as np
import ml_dtypes
from concourse import bacc, tile, mybir
from concourse.ap import AP
from concourse.bass_utils import run_bass_kernel_spmd

BF16 = mybir.dt.bfloat16
F32 = mybir.dt.float32
AF = mybir.ActivationFunctionType
ALU = mybir.AluOpType

N_CORES = 8
B = 32          # per-core batch
EPS = 1e-5

# l, cin, cout, hin (after any pool of prev layer), hout, pool_after
CONV = [
    (1, 1, 128, 28, 26, False),
    (2, 128, 128, 26, 24, False),
    (3, 128, 256, 24, 22, False),
    (4, 256, 256, 22, 20, True),
    (5, 256, 512, 10, 8, False),
    (6, 512, 512, 8, 6, True),
]
SPILL = {3, 4}

# layer-1 output-row groups for 4x PE row-group packing
L1_GROUPS = [(0, 7), (7, 6), (13, 7), (20, 6)]


def _halves(wout):
    """Split a row of wout*B output columns into <=512-col matmul chunks."""
    n = wout * B
    if n <= 512:
        return [(0, wout)]
    assert wout % 2 == 0
    return [(0, wout // 2), (wout // 2, wout // 2)]


def build(upto=9, dbg=True):
    """Build the Bass module. upto: 1..6 = stop after conv layer `upto` and
    emit its transformed activations as debug outputs (dbg=False: tiny output
    only, for timing bisection); 9 = full net."""
    nc = bacc.Bacc("TRN2", target_bir_lowering=False, debug=False,
                   num_devices=N_CORES)

    # ---- parameters (per-core shards / replicated weights)
    x_hi = nc.declare_dram_parameter("x_hi", [28 * 28, B], BF16, isOutput=False)
    x_lo = nc.declare_dram_parameter("x_lo", [28 * 28, B], BF16, isOutput=False)
    w1p = nc.declare_dram_parameter("w1p", [128, 128], BF16, isOutput=False)
    wp = {}
    gp, bp = {}, {}
    for (l, cin, cout, hin, hout, pool) in CONV:
        if l >= 2:
            ci_ch, co_ch = cin // 128, cout // 128
            wp[l] = nc.declare_dram_parameter(
                f"w{l}p", [ci_ch * 128, co_ch * 9 * 128], BF16, isOutput=False)
        gp[l] = nc.declare_dram_parameter(f"g{l}p", [cout, 1], F32, isOutput=False)
        bp[l] = nc.declare_dram_parameter(f"b{l}p", [cout, 1], F32, isOutput=False)
    fw1p = fw2p = fw3p = None
    if upto >= 9:
        fw1p = nc.declare_dram_parameter("fw1p", [36 * 128, 1024], BF16, isOutput=False)
        fw2p = nc.declare_dram_parameter("fw2p", [8 * 128, 1024], BF16, isOutput=False)
        fw3p = nc.declare_dram_parameter("fw3p", [8 * 128, 10], BF16, isOutput=False)
        out_ext = nc.declare_dram_parameter("out", [10, B], F32, isOutput=True)

    dbg_hi = dbg_lo = None
    if upto < 9:
        (_, _, cout, _, hout, pool) = CONV[upto - 1]
        ho = hout // 2 if pool else hout
        co_ch = cout // 128
        if dbg:
            dbg_hi = nc.declare_dram_parameter(
                "dbg_hi", [co_ch * 128, ho * ho * B], BF16, isOutput=True)
            dbg_lo = nc.declare_dram_parameter(
                "dbg_lo", [co_ch * 128, ho * ho * B], BF16, isOutput=True)
        else:
            dbg_hi = nc.declare_dram_parameter(
                "dbg_hi", [128, B], BF16, isOutput=True)

    # ---- DRAM scratch
    cc_in, cc_out = {}, {}
    for (l, cin, cout, hin, hout, pool) in CONV:
        if l > upto:
            break
        co_ch = cout // 128
        cc_in[l] = nc.dram_tensor(f"cc_in{l}", [128, 2 * co_ch], F32)
        cc_out[l] = nc.dram_tensor(f"cc_out{l}", [128, 2 * co_ch], F32,
                                   addr_space="Shared")
    raw_dram = {}
    for l in SPILL:
        if l > upto:
            continue
        (_, cin, cout, hin, hout, pool) = CONV[l - 1]
        co_ch = cout // 128
        raw_dram[l] = nc.dram_tensor(f"raw{l}d", [co_ch * hout * 128, hout * B], F32)

    with tile.TileContext(nc) as tc:
        const_pool = tc.alloc_tile_pool(name="const", bufs=1, side="left")
        stage_pool = tc.alloc_tile_pool(name="stage", bufs=1, side="right")
        psum_pool = tc.alloc_tile_pool(name="psum", bufs=6, space="PSUM")

        eps_t = const_pool.tile([128, 1], F32, tag="eps")
        nc.vector.memset(eps_t[:], EPS)

        # per-layer gamma/beta -> SBUF  [128,1] per chunk
        gb_sb = {}
        for (l, cin, cout, hin, hout, pool) in CONV:
            if l > upto:
                break
            co_ch = cout // 128
            for co in range(co_ch):
                gt = const_pool.tile([128, 1], F32, tag=f"g{l}_{co}")
                bt = const_pool.tile([128, 1], F32, tag=f"b{l}_{co}")
                nc.sync.dma_start(out=gt[:], in_=gp[l][co * 128:(co + 1) * 128, :])
                nc.sync.dma_start(out=bt[:], in_=bp[l][co * 128:(co + 1) * 128, :])
                gb_sb[(l, co)] = (gt, bt)

        # ---------------- conv stack ----------------
        def conv_layer(l, act_in, act_pool_in):
            """act_in: dict (ci, row) -> (hi_tile, lo_tile) of input rows
            [128, win*B] (layer 1: the packed im2col tiles under key (0, 0)).
            Returns (act_out, act_pool_out)."""
            (_, cin, cout, hin, hout, pool) = CONV[l - 1]
            ci_ch, co_ch = (cin // 128, cout // 128) if l >= 2 else (1, 1)
            spill = l in SPILL
            halves = _halves(hout)
            ntiles = hout * len(halves)

            # ---- right-stack transients (alloc order = reverse release order)
            wpool = tc.alloc_tile_pool(name=f"w{l}", bufs=1, side="right")
            stats_pool = tc.alloc_tile_pool(name=f"st{l}", bufs=1, side="right")
            raw_pool = None
            if not spill:
                raw_pool = tc.alloc_tile_pool(name=f"raw{l}", bufs=1, side="right")
            y32_pool = tc.alloc_tile_pool(name=f"y32_{l}", bufs=3, side="right")

            w_sb = {}
            if l >= 2:
                for ci in range(ci_ch):
                    for co in range(co_ch):
                        t = wpool.tile([128, 9 * 128], BF16, tag=f"w{ci}_{co}",
                                       name=f"w{l}_{ci}_{co}")
                        nc.sync.dma_start(
                            out=t[:],
                            in_=wp[l][ci * 128:(ci + 1) * 128,
                                      co * 1152:(co + 1) * 1152])
                        w_sb[(ci, co)] = t
            else:
                t = wpool.tile([128, 128], BF16, tag="w1", name="w1sb")
                nc.sync.dma_start(out=t[:], in_=w1p[:, :])
                w_sb[(0, 0)] = t

            stats = {}
            for co in range(co_ch):
                stats[co] = stats_pool.tile([128, ntiles * 6], F32, tag=f"s{co}",
                                            name=f"stats{l}_{co}")
            raw = {}
            if not spill:
                for co in range(co_ch):
                    for r in range(hout):
                        raw[(co, r)] = raw_pool.tile([128, hout * B], F32,
                                                     tag=f"r{co}_{r}",
                                                     name=f"raw{l}_{co}_{r}")

            def evac(ps, co, r, hidx, w0, wn):
                n = wn * B
                tidx = r * len(halves) + hidx
                nc.vector.bn_stats(
                    out=stats[co][:, tidx * 6:(tidx + 1) * 6], in_=ps[:])
                if spill:
                    sg = stage_pool.tile([128, n], F32, tag="evac", bufs=4,
                                         name="evac_sg")
                    nc.scalar.copy(out=sg[:], in_=ps[:])
                    nc.sync.dma_start(
                        out=raw_dram[l][(co * hout + r) * 128:
                                        (co * hout + r + 1) * 128,
                                        w0 * B:w0 * B + n],
                        in_=sg[:])
                else:
                    nc.scalar.copy(out=raw[(co, r)][:, w0 * B:w0 * B + n],
                                   in_=ps[:])

            # ---- matmuls
            if l == 1:
                ic_hi, ic_lo = act_in[(0, 0)]
                for gi, (r0, nr) in enumerate(L1_GROUPS):
                    for rr in range(nr):
                        for hidx, (w0, wn) in enumerate(halves):
                            n = wn * B
                            ps = psum_pool.tile([128, n], F32, tag="ps",
                                                name="ps1")
                            off = (rr * 26 + w0) * B
                            lhsT = w_sb[(0, 0)][32 * gi:32 * gi + 9, :]
                            nc.tensor.matmul(
                                out=ps[:], lhsT=lhsT,
                                rhs=ic_hi[32 * gi:32 * gi + 9, off:off + n],
                                start=True, stop=False,
                                tile_position=(32 * gi, 0))
                            nc.tensor.matmul(
                                out=ps[:], lhsT=lhsT,
                                rhs=ic_lo[32 * gi:32 * gi + 9, off:off + n],
                                start=False, stop=True,
                                tile_position=(32 * gi, 0))
                            evac(ps, 0, r0 + rr, hidx, w0, wn)
            else:
                for r in range(hout):
                    for co in range(co_ch):
                        for hidx, (w0, wn) in enumerate(halves):
                            n = wn * B
                            ps = psum_pool.tile([128, n], F32, tag="ps",
                                                name="psc")
                            nmm = ci_ch * 9 * 2
                            k = 0
                            for ci in range(ci_ch):
                                for dy in range(3):
                                    hi_t, lo_t = act_in[(ci, r + dy)]
                                    v_hi = hi_t[:].rearrange(
                                        "p (w b) -> p w b", b=B)
                                    v_lo = lo_t[:].rearrange(
                                        "p (w b) -> p w b", b=B)
                                    for dx in range(3):
                                        rhs_hi = v_hi[:, w0 + dx:w0 + dx + wn, :]
                                        rhs_lo = v_lo[:, w0 + dx:w0 + dx + wn, :]
                                        t = dy * 3 + dx
                                        lhsT = w_sb[(ci, co)][
                                            :, t * 128:(t + 1) * 128]
                                        nc.tensor.matmul(
                                            out=ps[:], lhsT=lhsT, rhs=rhs_hi,
                                            start=(k == 0), stop=False)
                                        k += 1
                                        nc.tensor.matmul(
                                            out=ps[:], lhsT=lhsT, rhs=rhs_lo,
                                            start=False, stop=(k == nmm - 1))
                                        k += 1
                            evac(ps, co, r, hidx, w0, wn)

            # ---- stats -> allreduce -> a, c
            cc_sb = const_pool.tile([128, 2 * co_ch], F32, tag=f"cc{l}",
                                    name=f"cc{l}")
            for co in range(co_ch):
                nc.vector.bn_aggr(out=cc_sb[:, co * 2:co * 2 + 2], in_=stats[co][:])
                # replace var slot with var + mean^2
                nc.vector.scalar_tensor_tensor(
                    out=cc_sb[:, co * 2 + 1:co * 2 + 2],
                    in0=cc_sb[:, co * 2:co * 2 + 1],
                    scalar=cc_sb[:, co * 2:co * 2 + 1],
                    in1=cc_sb[:, co * 2 + 1:co * 2 + 2],
                    op0=ALU.mult, op1=ALU.add)
            nc.sync.dma_start(out=cc_in[l][:, :], in_=cc_sb[:])
            nc.gpsimd.collective_compute(
                "AllReduce", ALU.add, replica_groups=[list(range(N_CORES))],
                ins=[cc_in[l][:, :]], outs=[cc_out[l][:, :]])
            cc_g = const_pool.tile([128, 2 * co_ch], F32, tag=f"ccg{l}",
                                   name=f"ccg{l}")
            nc.sync.dma_start(out=cc_g[:], in_=cc_out[l][:, :])

            ac = {}
            for co in range(co_ch):
                gt, bt = gb_sb[(l, co)]
                nm = const_pool.tile([128, 1], F32, tag=f"nm{l}_{co}",
                                     name=f"nm{l}_{co}")
                # nm = -mean_global = s1 * (-1/8)
                nc.scalar.mul(out=nm[:], in_=cc_g[:, co * 2:co * 2 + 1],
                              mul=-1.0 / N_CORES)
                nvar = const_pool.tile([128, 1], F32, tag=f"va{l}_{co}",
                                       name=f"nvar{l}_{co}")
                # nvar = mean^2 - s2/8 = -var
                nc.scalar.mul(out=nvar[:], in_=cc_g[:, co * 2 + 1:co * 2 + 2],
                              mul=1.0 / N_CORES)
                nc.vector.scalar_tensor_tensor(
                    out=nvar[:], in0=nm[:], scalar=nm[:], in1=nvar[:],
                    op0=ALU.mult, op1=ALU.subtract)
                sd = const_pool.tile([128, 1], F32, tag=f"sd{l}_{co}",
                                     name=f"sd{l}_{co}")
                nc.scalar.activation(out=sd[:], in_=nvar[:], func=AF.Sqrt,
                                     bias=eps_t[:], scale=-1.0)
                rc = const_pool.tile([128, 1], F32, tag=f"rc{l}_{co}",
                                     name=f"rc{l}_{co}")
                nc.vector.reciprocal(out=rc[:], in_=sd[:])
                a_t = const_pool.tile([128, 1], F32, tag=f"a{l}_{co}",
                                      name=f"a{l}_{co}")
                nc.vector.tensor_mul(out=a_t[:], in0=rc[:], in1=gt[:])
                c_t = const_pool.tile([128, 1], F32, tag=f"c{l}_{co}",
                                      name=f"c{l}_{co}")
                # c = b + nm * a
                nc.vector.scalar_tensor_tensor(
                    out=c_t[:], in0=nm[:], scalar=a_t[:], in1=bt[:],
                    op0=ALU.mult, op1=ALU.add)
                ac[co] = (a_t, c_t)

            # ---- input activations are dead; swap left-stack pools
            if act_pool_in is not None:
                act_pool_in.release()
            act_pool_out = tc.alloc_tile_pool(name=f"act{l}", bufs=1,
                                              side="left")

            # ---- transform (+ optional pool)
            ho = hout // 2 if pool else hout
            wo = hout // 2 if pool else hout
            act_out = {}
            for co in range(co_ch):
                a_t, c_t = ac[co]
                for r in range(ho):
                    r0 = r1 = src = None
                    if spill:
                        if pool:
                            r0 = stage_pool.tile([128, hout * B], F32,
                                                 tag="rb0", bufs=2, name="rb0")
                            r1 = stage_pool.tile([128, hout * B], F32,
                                                 tag="rb1", bufs=2, name="rb1")
                            nc.sync.dma_start(
                                out=r0[:], in_=raw_dram[l][
                                    (co * hout + 2 * r) * 128:
                                    (co * hout + 2 * r + 1) * 128, :])
                            nc.sync.dma_start(
                                out=r1[:], in_=raw_dram[l][
                                    (co * hout + 2 * r + 1) * 128:
                                    (co * hout + 2 * r + 2) * 128, :])
                        else:
                            r0 = stage_pool.tile([128, hout * B], F32,
                                                 tag="rb0", bufs=2, name="rb0")
                            nc.sync.dma_start(
                                out=r0[:], in_=raw_dram[l][
                                    (co * hout + r) * 128:
                                    (co * hout + r + 1) * 128, :])
                            src = r0
                    else:
                        if pool:
                            r0, r1 = raw[(co, 2 * r)], raw[(co, 2 * r + 1)]
                        else:
                            src = raw[(co, r)]
                    if pool:
                        pm = stage_pool.tile([128, hout * B], F32, tag="pm",
                                             bufs=2, name="pm")
                        nc.vector.tensor_max(out=pm[:], in0=r0[:], in1=r1[:])
                        v = pm[:].rearrange("p (w two b) -> p w two b",
                                            two=2, b=B)
                        pr = stage_pool.tile([128, wo * B], F32, tag="pr",
                                             bufs=2, name="pr")
                        prv = pr[:].rearrange("p (w b) -> p w b", b=B)
                        nc.vector.tensor_max(out=prv[:, :, :],
                                             in0=v[:, :, 0, :],
                                             in1=v[:, :, 1, :])
                        src = pr
                    hi_t = act_pool_out.tile([128, wo * B], BF16,
                                             tag=f"h{co}_{r}",
                                             name=f"a{l}h_{co}_{r}")
                    lo_t = act_pool_out.tile([128, wo * B], BF16,
                                             tag=f"l{co}_{r}",
                                             name=f"a{l}l_{co}_{r}")
                    nc.scalar.activation(out=hi_t[:], in_=src[:], func=AF.Relu,
                                         bias=c_t[:], scale=a_t[:])
                    y32 = y32_pool.tile([128, wo * B], F32, tag="y32",
                                        name="y32t")
                    nc.scalar.activation(out=y32[:], in_=src[:], func=AF.Relu,
                                         bias=c_t[:], scale=a_t[:])
                    nc.vector.tensor_sub(out=lo_t[:], in0=y32[:], in1=hi_t[:])
                    act_out[(co, r)] = (hi_t, lo_t)

            # ---- pop right-stack transients (LIFO)
            y32_pool.release()
            if raw_pool is not None:
                raw_pool.release()
            stats_pool.release()
            wpool.release()
            return act_out, act_pool_out

        # ---- layer 1 im2col source (4 row-groups packed on partitions)
        im2col_pool = tc.alloc_tile_pool(name="im2col", bufs=1, side="left")
        ic_hi = im2col_pool.tile([128, 7 * 26 * B], BF16, tag="ic_hi")
        ic_lo = im2col_pool.tile([128, 7 * 26 * B], BF16, tag="ic_lo")
        for src, dst in ((x_hi, ic_hi), (x_lo, ic_lo)):
            for gi, (r0, nr) in enumerate(L1_GROUPS):
                for dy in range(3):
                    # partitions 32*gi + 3*dy + dx <- x[(r0+ho+dy)*28 + wo+dx, b]
                    in_ap = AP(src, ((r0 + dy) * 28) * B,
                               [[B, 3], [28 * B, nr], [B, 26], [1, B]])
                    nc.sync.dma_start(
                        out=dst[32 * gi + 3 * dy:32 * gi + 3 * dy + 3,
                                0:nr * 26 * B],
                        in_=in_ap)

        act = {(0, 0): (ic_hi, ic_lo)}
        act, act_pool = conv_layer(1, act, im2col_pool)

        for l in range(2, min(upto, 6) + 1):
            act, act_pool = conv_layer(l, act, act_pool)

        if upto < 9:
            # dump act (hi, lo) of last computed layer
            (_, _, cout, _, hout, pool) = CONV[upto - 1]
            ho = hout // 2 if pool else hout
            co_ch = cout // 128
            if not dbg:
                hi_t, _ = act[(0, 0)]
                nc.sync.dma_start(out=dbg_hi[:, :], in_=hi_t[:, :B])
            else:
                for co in range(co_ch):
                    for r in range(ho):
                        hi_t, lo_t = act[(co, r)]
                        nc.sync.dma_start(
                            out=dbg_hi[co * 128:(co + 1) * 128,
                                       r * ho * B:(r + 1) * ho * B],
                            in_=hi_t[:])
                        nc.sync.dma_start(
                            out=dbg_lo[co * 128:(co + 1) * 128,
                                       r * ho * B:(r + 1) * ho * B],
                            in_=lo_t[:])
        else:
            # ---------------- FC stack ----------------
            fc_pool = tc.alloc_tile_pool(name="fc", bufs=1, side="right")
            fw1_sb = []
            for kc in range(36):
                t = fc_pool.tile([128, 1024], BF16, tag=f"fw1_{kc}",
                                 name=f"fw1_{kc}")
                nc.sync.dma_start(out=t[:],
                                  in_=fw1p[kc * 128:(kc + 1) * 128, :])
                fw1_sb.append(t)
            fw2_sb = []
            for kc in range(8):
                t = fc_pool.tile([128, 1024], BF16, tag=f"fw2_{kc}",
                                 name=f"fw2_{kc}")
                nc.sync.dma_start(out=t[:],
                                  in_=fw2p[kc * 128:(kc + 1) * 128, :])
                fw2_sb.append(t)
            fw3_sb = []
            for kc in range(8):
                t = fc_pool.tile([128, 10], BF16, tag=f"fw3_{kc}",
                                 name=f"fw3_{kc}")
                nc.sync.dma_start(out=t[:],
                                  in_=fw3p[kc * 128:(kc + 1) * 128, :])
                fw3_sb.append(t)

            def fc_relu_layer(w_sb, rhs_list, n_out_ch, tagp):
                """rhs_list: list over kc of (hi_ap, lo_ap) [128, B].
                Returns list over o_ch of (hi, lo) tiles [128, B]."""
                outs = []
                nkc = len(rhs_list)
                for o_ch in range(n_out_ch):
                    ps = psum_pool.tile([128, B], F32, tag="ps", name="psfc")
                    for kc, (rh, rl) in enumerate(rhs_list):
                        lhsT = w_sb[kc][:, o_ch * 128:(o_ch + 1) * 128]
                        nc.tensor.matmul(out=ps[:], lhsT=lhsT, rhs=rh,
                                         start=(kc == 0), stop=False)
                        nc.tensor.matmul(out=ps[:], lhsT=lhsT, rhs=rl,
                                         start=False, stop=(kc == nkc - 1))
                    hi_t = fc_pool.tile([128, B], BF16, tag=f"{tagp}h{o_ch}",
                                        name=f"{tagp}h{o_ch}")
                    lo_t = fc_pool.tile([128, B], BF16, tag=f"{tagp}l{o_ch}",
                                        name=f"{tagp}l{o_ch}")
                    nc.scalar.activation(out=hi_t[:], in_=ps[:], func=AF.Relu)
                    y32 = fc_pool.tile([128, B], F32, tag=f"{tagp}y{o_ch}",
                                       name=f"{tagp}y{o_ch}")
                    nc.scalar.activation(out=y32[:], in_=ps[:], func=AF.Relu)
                    nc.vector.tensor_sub(out=lo_t[:], in0=y32[:], in1=hi_t[:])
                    outs.append((hi_t, lo_t))
                return outs

            # fc1 rhs: act6 [(co,r)] tiles [128, 3*B]; kc = (co*3 + r)*3 + w
            rhs1 = []
            for co in range(4):
                for r in range(3):
                    hi_t, lo_t = act[(co, r)]
                    for w in range(3):
                        rhs1.append((hi_t[:, w * B:(w + 1) * B],
                                     lo_t[:, w * B:(w + 1) * B]))
            y1 = fc_relu_layer(fw1_sb, rhs1, 8, "y1")
            y2 = fc_relu_layer(fw2_sb, [(h[:], l_[:]) for h, l_ in y1], 8, "y2")

            ps = psum_pool.tile([10, B], F32, tag="ps", name="ps10")
            for kc, (rh, rl) in enumerate([(h[:], l_[:]) for h, l_ in y2]):
                lhsT = fw3_sb[kc][:, :]
                nc.tensor.matmul(out=ps[:], lhsT=lhsT, rhs=rh,
                                 start=(kc == 0), stop=False)
                nc.tensor.matmul(out=ps[:], lhsT=lhsT, rhs=rl,
                                 start=False, stop=(kc == 7))
            sig = fc_pool.tile([10, B], F32, tag="sig", name="sig")
            nc.scalar.activation(out=sig[:], in_=ps[:], func=AF.Sigmoid)
            nc.sync.dma_start(out=out_ext[:, :], in_=sig[:])
            fc_pool.release()

        act_pool.release()
        psum_pool.release()
        stage_pool.release()
        const_pool.release()

    nc.compile()
    return nc


# ---------------- host-side input prep ----------------

def _bf16(x):
    return np.asarray(x, np.float32).astype(ml_dtypes.bfloat16)


def prep_inputs(inputs):
    """Full reference inputs -> per-core input maps."""
    x = np.asarray(inputs["x"], np.float32).reshape(256, 28 * 28)
    shared = {}
    w1 = np.sign(np.asarray(inputs["w1"], np.float32)).reshape(128, 9)
    w1rep = np.zeros((128, 128), np.float32)
    for gi in range(4):
        w1rep[32 * gi:32 * gi + 9, :] = w1.T
    shared["w1p"] = _bf16(w1rep)
    for l in (2, 3, 4, 5, 6):
        w = np.sign(np.asarray(inputs[f"w{l}"], np.float32))
        cout, cin = w.shape[0], w.shape[1]
        ci_ch, co_ch = cin // 128, cout // 128
        s = w.reshape(co_ch, 128, ci_ch, 128, 9)
        s = np.transpose(s, (2, 3, 0, 4, 1))  # [ci_ch,128,co_ch,9,128]
        shared[f"w{l}p"] = _bf16(s.reshape(ci_ch * 128, co_ch * 9 * 128).copy())
    for l in range(1, 7):
        shared[f"g{l}p"] = np.asarray(inputs[f"g{l}"], np.float32).reshape(-1, 1)
        shared[f"b{l}p"] = np.asarray(inputs[f"b{l}"], np.float32).reshape(-1, 1)
    fw1 = np.sign(np.asarray(inputs["fw1"], np.float32))  # [1024, 4608]
    v = fw1.reshape(1024, 4, 128, 9)
    v = np.transpose(v, (1, 3, 2, 0))  # [4, 9, 128, 1024], kc = cc*9+hw
    shared["fw1p"] = _bf16(v.reshape(36 * 128, 1024).copy())
    fw2 = np.sign(np.asarray(inputs["fw2"], np.float32))
    shared["fw2p"] = _bf16(fw2.T.reshape(8 * 128, 1024).copy())
    fw3 = np.sign(np.asarray(inputs["fw3"], np.float32))
    shared["fw3p"] = _bf16(fw3.T.reshape(8 * 128, 10).copy())

    per_core = []
    for c in range(N_CORES):
        xs = x[c * B:(c + 1) * B].T.copy()  # [784, B]
        hi = _bf16(xs)
        lo = _bf16(xs - hi.astype(np.float32))
        m = dict(shared)
        m["x_hi"] = hi
        m["x_lo"] = lo
        per_core.append(m)
    return per_core


_NC_CACHE = {}


def kernel(**inputs) -> np.ndarray:
    if "nc" not in _NC_CACHE:
        _NC_CACHE["nc"] = build(upto=9)
    nc = _NC_CACHE["nc"]
    per_core = prep_inputs(inputs)
    last_err = None
    for _attempt in range(3):
        try:
            res = run_bass_kernel_spmd(nc, per_core, list(range(N_CORES)))
            break
        except Exception as e:  # transient device wedge: wait and retry
            last_err = e
            import time as _time
            _time.sleep(15)
    else:
        raise last_err
    out = np.empty((256, 10), np.float32)
    for c in range(N_CORES):
        out[c * B:(c + 1) * B] = res.results[c]["out"].T
    return out


if __name__ == "__main__":
    import reference as R
    inputs = R.setup_inputs()
    got = kernel(**{k: np.asarray(v) for k, v in inputs.items()})
    exp = np.asarray(R.reference(**inputs))
    err = np.abs(got - exp)
    rel = np.linalg.norm(got - exp) / np.linalg.norm(exp)
    print(f"absmax {err.max():.3e}  rel {rel:.3e}")
